# revision 6
# baseline (speedup 1.0000x reference)
"""Trainium2 Bass kernel for gnn_message_passing (nn_Conv_82506321756833).

Computes, for N=50000 nodes / E=800000 edges / H=128:
    xp   = gelu(x @ W1 + b1)
    aggr = segment_sum(xp[src] * bases, dst)
    x    = x_feat + aggr
    y    = gelu(bn1(x @ W2 + b2)); y = gelu(bn2(y @ W3 + b3))
    out  = x + y

Sharding: nodes are partitioned contiguously across 8 cores (graph parallel);
each core owns its node shard and all edges whose dst lands in the shard.
Every core redundantly computes the full xp table (cheap GEMM) so the edge
gather is purely local.  The scatter-sum is done on-chip with one-hot
matmuls: edges are bucketed by 128-node destination windows (host-side sort),
and for each 128-edge tile a one-hot matrix S[e, slot] selects the
destination slot; PE computes msg.T @ S accumulating into PSUM (feature-major
output feeds the FFN directly, with BN folded into W2/W3 + bias vectors).

The edge gather uses InstDMAGatherAnt (int16 indices, signed): each window's
edges are split into "lo" (src < 32768) and "hi" tile groups; the hi group
gathers from an offset view of the xp table.
"""

import numpy as np

import concourse.bass as bass
import concourse.bacc as bacc
import concourse.tile as tile
from concourse import mybir
from concourse.bass_utils import run_bass_kernel_spmd

H = 128
WIN = 128
SPLIT = 32768  # int16 index limit for dma_gather
BN_EPS = 1e-5
F16 = mybir.dt.float16
F32 = mybir.dt.float32
I16 = mybir.dt.int16
GELU = mybir.ActivationFunctionType.Gelu


def _ceil_to(x, m):
    return (x + m - 1) // m * m


def _wrap16(idx, nw, L):
    """[nw, L] int16 index lists -> [nw*128, L//16] wrapped+replicated."""
    m = idx.reshape(nw, L // 16, 16).transpose(0, 2, 1)  # [nw, 16, L/16]
    m = np.tile(m, (1, 8, 1))                            # [nw, 128, L/16]
    return np.ascontiguousarray(m.reshape(nw * 128, L // 16))


def prep_inputs(x_feat, bases, src, dst, W1, b1, W2, b2, W3, b3,
                g1, be1, m1, v1, g2, be2, m2, v2, ncores=8):
    """Host-side sharding: bucket edges by (dst window, src-range), sort,
    pad each group to a fixed tile count, build per-core input maps."""
    N = x_feat.shape[0]
    assert N % ncores == 0
    NSH = N // ncores
    NW = (NSH + WIN - 1) // WIN
    NPAD = NW * WIN
    NA = _ceil_to(N, 128)

    x_feat = np.asarray(x_feat, np.float32)
    bases = np.asarray(bases, np.float32)
    src = np.asarray(src, np.int64)
    dst = np.asarray(dst, np.int64)

    xT = np.zeros((H, NA), np.float16)
    xT[:, :N] = x_feat.T

    core_of = dst // NSH
    percore = []
    TLO = THI = 0
    for k in range(ncores):
        sel = np.nonzero(core_of == k)[0]
        ld = dst[sel] - k * NSH
        w = ld // WIN
        j = ld % WIN
        s = src[sel]
        hi = (s >= SPLIT).astype(np.int64)
        key2 = w * 2 + hi
        order = np.lexsort((s, key2))
        w, j, s, hi, key2, sel = (w[order], j[order], s[order], hi[order],
                                  key2[order], sel[order])
        cnt2 = np.bincount(key2, minlength=NW * 2)
        nlo = cnt2[0::2]
        nhi = cnt2[1::2]
        TLO = max(TLO, int(np.max((nlo + 127) // 128)))
        THI = max(THI, int(np.max((nhi + 127) // 128)))
        starts2 = np.zeros(NW * 2, np.int64)
        np.cumsum(cnt2[:-1], out=starts2[1:])
        rank = np.arange(len(w)) - starts2[key2]
        percore.append((w, j, s, hi, rank, sel))
    TLO = max(TLO, 1)
    T = TLO + THI

    w1h = np.ascontiguousarray(np.asarray(W1, np.float32).astype(np.float16))
    a1 = (np.asarray(g1, np.float32) /
          np.sqrt(np.asarray(v1, np.float32) + BN_EPS))
    a2 = (np.asarray(g2, np.float32) /
          np.sqrt(np.asarray(v2, np.float32) + BN_EPS))
    w2f = np.ascontiguousarray((np.asarray(W2, np.float32) * a1[None, :])
                               .astype(np.float16))
    w3f = np.ascontiguousarray((np.asarray(W3, np.float32) * a2[None, :])
                               .astype(np.float16))
    c2 = ((np.asarray(b2, np.float32) - np.asarray(m1, np.float32)) * a1
          + np.asarray(be1, np.float32)).astype(np.float32).reshape(H, 1)
    c3 = ((np.asarray(b3, np.float32) - np.asarray(m2, np.float32)) * a2
          + np.asarray(be2, np.float32)).astype(np.float32).reshape(H, 1)
    have_b1 = bool(np.any(np.asarray(b1)))
    b1h = np.asarray(b1, np.float32).astype(np.float16).reshape(1, H)

    in_maps = []
    for k in range(ncores):
        w, j, s, hi, rank, sel = percore[k]
        # position of each edge inside its window's [T*128] slot grid:
        # lo edges occupy tiles [0, TLO), hi edges tiles [TLO, T).
        pos = np.where(hi == 1, TLO * 128 + rank, rank)
        # dma_gather writes index i -> (partition i%128, block i//128)
        perm = (w * 128 + pos % 128) * T + pos // 128
        bas_all = np.zeros((NW * 128 * T, H), np.float16)
        bas_all[perm] = bases[sel].astype(np.float16)
        j_all = np.zeros(NW * 128 * T, np.float32)
        j_all[perm] = j.astype(np.float32)

        ilo = np.zeros((NW, TLO * 128), np.int16)
        lo_m = hi == 0
        ilo[w[lo_m], rank[lo_m]] = s[lo_m].astype(np.int16)
        maps = dict(
            xT=xT,
            basd=bas_all.reshape(NW * 128, T * H),
            jd=np.ascontiguousarray(j_all.reshape(NW * 128, T)),
            ilod=_wrap16(ilo, NW, TLO * 128),
            w1=w1h, w2=w2f, w3=w3f, c2=c2, c3=c3,
        )
        if THI:
            ihi = np.zeros((NW, THI * 128), np.int16)
            hi_m = hi == 1
            ihi[w[hi_m], rank[hi_m]] = (s[hi_m] - SPLIT).astype(np.int16)
            maps["ihid"] = _wrap16(ihi, NW, THI * 128)
        xfm = np.zeros((H, NPAD), np.float32)
        xfm[:, :NSH] = x_feat[k * NSH:(k + 1) * NSH].T
        maps["xfm"] = xfm
        if have_b1:
            maps["b1"] = b1h
        in_maps.append(maps)
    meta = dict(N=N, NSH=NSH, NW=NW, NPAD=NPAD, NA=NA,
                TLO=TLO, THI=THI, T=T, have_b1=have_b1)
    return in_maps, meta


def build_program(meta, ncores=8, act=GELU):
    NA, NW, NPAD = meta["NA"], meta["NW"], meta["NPAD"]
    TLO, THI, T = meta["TLO"], meta["THI"], meta["T"]
    have_b1 = meta["have_b1"]
    EPW = T * H

    nc = bacc.Bacc("TRN2", target_bir_lowering=False, debug=False,
                   num_devices=ncores)
    xT = nc.dram_tensor("xT", [H, NA], F16, kind="ExternalInput").ap()
    xfm = nc.dram_tensor("xfm", [H, NPAD], F32, kind="ExternalInput").ap()
    basd = nc.dram_tensor("basd", [NW * 128, EPW], F16,
                          kind="ExternalInput").ap()
    jd = nc.dram_tensor("jd", [NW * 128, T], F32, kind="ExternalInput").ap()
    ilod = nc.dram_tensor("ilod", [NW * 128, TLO * 8], I16,
                          kind="ExternalInput").ap()
    ihid = (nc.dram_tensor("ihid", [NW * 128, THI * 8], I16,
                           kind="ExternalInput").ap() if THI else None)
    w1 = nc.dram_tensor("w1", [H, H], F16, kind="ExternalInput").ap()
    w2 = nc.dram_tensor("w2", [H, H], F16, kind="ExternalInput").ap()
    w3 = nc.dram_tensor("w3", [H, H], F16, kind="ExternalInput").ap()
    c2 = nc.dram_tensor("c2", [H, 1], F32, kind="ExternalInput").ap()
    c3 = nc.dram_tensor("c3", [H, 1], F32, kind="ExternalInput").ap()
    b1 = (nc.dram_tensor("b1", [1, H], F16, kind="ExternalInput").ap()
          if have_b1 else None)
    outd = nc.dram_tensor("out", [H, NPAD], F32, kind="ExternalOutput").ap()
    xp_h = nc.dram_tensor("xp", [NA, H], F16)
    xp_full = xp_h.ap()

    with tile.TileContext(nc) as tc:
        with (
            tc.tile_pool(name="const", bufs=1) as cpool,
            tc.tile_pool(name="xa", bufs=3) as xa,
            tc.tile_pool(name="xo", bufs=3) as xo,
            tc.tile_pool(name="pa", bufs=2, space="PSUM") as pa,
            tc.tile_pool(name="bas", bufs=3) as basp,
            tc.tile_pool(name="gat", bufs=3) as gatp,
            tc.tile_pool(name="st", bufs=2) as stp,
            tc.tile_pool(name="small", bufs=3) as smallp,
            tc.tile_pool(name="ffn", bufs=2) as ffnp,
            tc.tile_pool(name="pag", bufs=2, space="PSUM") as pag,
            tc.tile_pool(name="pffn", bufs=2, space="PSUM") as pffn,
        ):
            # constants
            w1t = cpool.tile([H, H], F16, tag="w1")
            nc.sync.dma_start(w1t[:], w1[:])
            w2t = cpool.tile([H, H], F16, tag="w2")
            nc.sync.dma_start(w2t[:], w2[:])
            w3t = cpool.tile([H, H], F16, tag="w3")
            nc.sync.dma_start(w3t[:], w3[:])
            c2t = cpool.tile([H, 1], F32, tag="c2")
            nc.sync.dma_start(c2t[:], c2[:])
            c3t = cpool.tile([H, 1], F32, tag="c3")
            nc.sync.dma_start(c3t[:], c3[:])
            iot = cpool.tile([H, H], I16, tag="iota")
            nc.gpsimd.iota(iot[:], [[1, H]], channel_multiplier=0)
            if have_b1:
                b1t = cpool.tile([1, H], F16, tag="b1")
                nc.sync.dma_start(b1t[:], b1[:])
                onest = cpool.tile([1, H], F16, tag="ones")
                nc.gpsimd.memset(onest[:], 1.0)

            # ---- Phase A: xp = gelu(x @ W1 [+ b1]), node-major fp16 ----
            CH = 512
            for c0 in range(0, NA, CH):
                cols = min(CH, NA - c0)
                nb = cols // 128
                xt_t = xa.tile([H, CH], F16, tag="xa")
                nc.sync.dma_start(xt_t[:, :cols], xT[:, c0:c0 + cols])
                ps = pa.tile([128, CH], F32, tag="pa")
                for b in range(nb):
                    nc.tensor.matmul(
                        ps[:, b * 128:(b + 1) * 128],
                        xt_t[:, b * 128:(b + 1) * 128],
                        w1t[:],
                        start=True, stop=not have_b1)
                    if have_b1:
                        nc.tensor.matmul(
                            ps[:, b * 128:(b + 1) * 128],
                            onest[:1, :], b1t[:1, :],
                            start=False, stop=True)
                xo_t = xo.tile([128, CH], F16, tag="xo")
                nc.scalar.activation(xo_t[:, :cols], ps[:, :cols], act)
                dst_ap = bass.AP(xp_h, c0 * H,
                                 [[H, 128], [128 * H, nb], [1, H]])
                nc.sync.dma_start(dst_ap, xo_t[:, :cols])

            # ---- Phase B: gather, multiply, one-hot scatter, FFN ----
            for w in range(NW):
                r0 = w * 128
                bas_t = basp.tile([128, EPW], F16, tag="bas")
                nc.scalar.dma_start(bas_t[:], basd[r0:r0 + 128, :])
                il_t = smallp.tile([128, TLO * 8], I16, tag="il")
                nc.scalar.dma_start(il_t[:], ilod[r0:r0 + 128, :])
                if THI:
                    ih_t = smallp.tile([128, THI * 8], I16, tag="ih")
                    nc.scalar.dma_start(ih_t[:], ihid[r0:r0 + 128, :])
                j_t = smallp.tile([128, T], F32, tag="j")
                nc.scalar.dma_start(j_t[:], jd[r0:r0 + 128, :])

                g_t = gatp.tile([128, EPW], F16, tag="gat")
                g3 = g_t[:].rearrange("p (t f) -> p t f", f=H)
                nc.gpsimd.dma_gather(g3[:, 0:TLO, :], xp_full, il_t[:],
                                     TLO * 128, TLO * 128, H,
                                     single_packet=False)
                if THI:
                    nc.gpsimd.dma_gather(g3[:, TLO:T, :],
                                         xp_full[SPLIT:NA, :], ih_t[:],
                                         THI * 128, THI * 128, H,
                                         single_packet=False)
                msg_t = gatp.tile([128, EPW], F16, tag="msg")
                nc.vector.tensor_mul(msg_t[:], g_t[:], bas_t[:])

                s_t = stp.tile([128, EPW], F16, tag="s")
                for t in range(T):
                    nc.vector.tensor_scalar(
                        s_t[:, t * 128:(t + 1) * 128], iot[:],
                        j_t[:, t:t + 1], None, mybir.AluOpType.is_equal)

                ps_ag = pag.tile([128, 128], F32, tag="pag")
                for t in range(T):
                    nc.tensor.matmul(
                        ps_ag[:],
                        msg_t[:, t * 128:(t + 1) * 128],
                        s_t[:, t * 128:(t + 1) * 128],
                        start=(t == 0), stop=(t == T - 1))

                xf_t = smallp.tile([128, 128], F32, tag="xf")
                nc.scalar.dma_start(xf_t[:], xfm[:, r0:r0 + 128])
                x32_t = ffnp.tile([128, 128], F32, tag="x32")
                nc.vector.tensor_add(x32_t[:], ps_ag[:], xf_t[:])
                x16_t = ffnp.tile([128, 128], F16, tag="x16")
                nc.vector.tensor_copy(x16_t[:], x32_t[:])

                ps2 = pffn.tile([128, 128], F32, tag="pffn")
                nc.tensor.matmul(ps2[:], w2t[:], x16_t[:],
                                 start=True, stop=True)
                y1_t = ffnp.tile([128, 128], F16, tag="y1")
                nc.scalar.activation(y1_t[:], ps2[:], act, bias=c2t[:, 0:1])
                ps3 = pffn.tile([128, 128], F32, tag="pffn")
                nc.tensor.matmul(ps3[:], w3t[:], y1_t[:],
                                 start=True, stop=True)
                y2_t = ffnp.tile([128, 128], F32, tag="y2")
                nc.scalar.activation(y2_t[:], ps3[:], act, bias=c3t[:, 0:1])
                o_t = ffnp.tile([128, 128], F32, tag="o")
                nc.vector.tensor_add(o_t[:], y2_t[:], x32_t[:])
                nc.sync.dma_start(outd[:, r0:r0 + 128], o_t[:])

    nc.compile()
    return nc


def run_compiled(nc, in_maps, meta, ncores=8, **kw):
    res = run_bass_kernel_spmd(nc, in_maps, list(range(ncores)), **kw)
    N, NSH = meta["N"], meta["NSH"]
    out = np.empty((N, H), np.float32)
    for k in range(ncores):
        out[k * NSH:(k + 1) * NSH] = res.results[k]["out"][:, :NSH].T
    return out, res


def kernel(**inputs):
    inputs = {k: np.asarray(v) for k, v in inputs.items()}
    in_maps, meta = prep_inputs(**inputs)
    nc = build_program(meta)
    out, _ = run_compiled(nc, in_maps, meta)
    return out


# revision 7
# speedup vs baseline: 1.2499x; 1.2499x over previous
"""Trainium2 Bass kernel for gnn_message_passing (nn_Conv_82506321756833).

Computes, for N=50000 nodes / E=800000 edges / H=128:
    xp   = gelu(x @ W1 + b1)
    aggr = segment_sum(xp[src] * bases, dst)
    x    = x_feat + aggr
    y    = gelu(bn1(x @ W2 + b2)); y = gelu(bn2(y @ W3 + b3))
    out  = x + y

Sharding: nodes are partitioned contiguously across 8 cores (graph parallel);
each core owns its node shard and all edges whose dst lands in the shard.
Every core redundantly computes the full xp table (cheap GEMM) so the edge
gather is purely local.  The scatter-sum is done on-chip with one-hot
matmuls: edges are bucketed by 128-node destination windows (host-side sort),
and for each 128-edge tile a one-hot matrix S[e, slot] selects the
destination slot; PE computes msg.T @ S accumulating into PSUM (feature-major
output feeds the FFN directly, with BN folded into W2/W3 + bias vectors).

The edge gather uses InstDMAGatherAnt (int16 indices, signed): each window's
edges are split into "lo" (src < 32768) and "hi" tile groups; the hi group
gathers from an offset view of the xp table.
"""

import numpy as np

import concourse.bass as bass
import concourse.bacc as bacc
import concourse.tile as tile
from concourse import mybir
from concourse.bass_utils import run_bass_kernel_spmd

H = 128
WIN = 128
SPLIT = 32768  # int16 index limit for dma_gather
BN_EPS = 1e-5
F16 = mybir.dt.float16
F32 = mybir.dt.float32
I16 = mybir.dt.int16
GELU = mybir.ActivationFunctionType.Gelu


def _ceil_to(x, m):
    return (x + m - 1) // m * m


def _wrap16(idx, nw, L):
    """[nw, L] int16 index lists -> [nw*128, L//16] wrapped+replicated."""
    m = idx.reshape(nw, L // 16, 16).transpose(0, 2, 1)  # [nw, 16, L/16]
    m = np.tile(m, (1, 8, 1))                            # [nw, 128, L/16]
    return np.ascontiguousarray(m.reshape(nw * 128, L // 16))


def prep_inputs(x_feat, bases, src, dst, W1, b1, W2, b2, W3, b3,
                g1, be1, m1, v1, g2, be2, m2, v2, ncores=8):
    """Host-side sharding: bucket edges by (dst window, src-range), sort,
    pad each group to a fixed tile count, build per-core input maps."""
    N = x_feat.shape[0]
    assert N % ncores == 0
    NSH = N // ncores
    NW = (NSH + WIN - 1) // WIN
    NPAD = NW * WIN
    NA = _ceil_to(N, 128)

    x_feat = np.asarray(x_feat, np.float32)
    bases = np.asarray(bases, np.float32)
    src = np.asarray(src, np.int64)
    dst = np.asarray(dst, np.int64)

    xT = np.zeros((H, NA), np.float16)
    xT[:, :N] = x_feat.T

    core_of = dst // NSH
    percore = []
    TLO = THI = 0
    for k in range(ncores):
        sel = np.nonzero(core_of == k)[0]
        ld = dst[sel] - k * NSH
        w = ld // WIN
        j = ld % WIN
        s = src[sel]
        hi = (s >= SPLIT).astype(np.int64)
        key2 = w * 2 + hi
        order = np.lexsort((s, key2))
        w, j, s, hi, key2, sel = (w[order], j[order], s[order], hi[order],
                                  key2[order], sel[order])
        cnt2 = np.bincount(key2, minlength=NW * 2)
        nlo = cnt2[0::2]
        nhi = cnt2[1::2]
        TLO = max(TLO, int(np.max((nlo + 127) // 128)))
        THI = max(THI, int(np.max((nhi + 127) // 128)))
        starts2 = np.zeros(NW * 2, np.int64)
        np.cumsum(cnt2[:-1], out=starts2[1:])
        rank = np.arange(len(w)) - starts2[key2]
        percore.append((w, j, s, hi, rank, sel))
    TLO = max(TLO, 1)
    T = TLO + THI

    w1h = np.ascontiguousarray(np.asarray(W1, np.float32).astype(np.float16))
    a1 = (np.asarray(g1, np.float32) /
          np.sqrt(np.asarray(v1, np.float32) + BN_EPS))
    a2 = (np.asarray(g2, np.float32) /
          np.sqrt(np.asarray(v2, np.float32) + BN_EPS))
    w2f = np.ascontiguousarray((np.asarray(W2, np.float32) * a1[None, :])
                               .astype(np.float16))
    w3f = np.ascontiguousarray((np.asarray(W3, np.float32) * a2[None, :])
                               .astype(np.float16))
    c2 = ((np.asarray(b2, np.float32) - np.asarray(m1, np.float32)) * a1
          + np.asarray(be1, np.float32)).astype(np.float32).reshape(H, 1)
    c3 = ((np.asarray(b3, np.float32) - np.asarray(m2, np.float32)) * a2
          + np.asarray(be2, np.float32)).astype(np.float32).reshape(H, 1)
    have_b1 = bool(np.any(np.asarray(b1)))
    b1h = np.asarray(b1, np.float32).astype(np.float16).reshape(1, H)

    in_maps = []
    for k in range(ncores):
        w, j, s, hi, rank, sel = percore[k]
        # position of each edge inside its window's [T*128] slot grid:
        # lo edges occupy tiles [0, TLO), hi edges tiles [TLO, T).
        pos = np.where(hi == 1, TLO * 128 + rank, rank)
        # dma_gather writes index i -> (partition i%128, block i//128)
        perm = (w * 128 + pos % 128) * T + pos // 128
        bas_all = np.zeros((NW * 128 * T, H), np.float16)
        bas_all[perm] = bases[sel].astype(np.float16)
        s_all = np.zeros((NW * 128 * T, H), np.float16)
        s_all[perm, j] = 1.0

        ilo = np.zeros((NW, TLO * 128), np.int16)
        lo_m = hi == 0
        ilo[w[lo_m], rank[lo_m]] = s[lo_m].astype(np.int16)
        maps = dict(
            xT=xT,
            basd=bas_all.reshape(NW * 128, T * H),
            sd=s_all.reshape(NW * 128, T * H),
            ilod=_wrap16(ilo, NW, TLO * 128),
            w1=w1h, w2=w2f, w3=w3f, c2=c2, c3=c3,
        )
        if THI:
            ihi = np.zeros((NW, THI * 128), np.int16)
            hi_m = hi == 1
            ihi[w[hi_m], rank[hi_m]] = (s[hi_m] - SPLIT).astype(np.int16)
            maps["ihid"] = _wrap16(ihi, NW, THI * 128)
        xfm = np.zeros((H, NPAD), np.float32)
        xfm[:, :NSH] = x_feat[k * NSH:(k + 1) * NSH].T
        maps["xfm"] = xfm
        if have_b1:
            maps["b1"] = b1h
        in_maps.append(maps)
    meta = dict(N=N, NSH=NSH, NW=NW, NPAD=NPAD, NA=NA,
                TLO=TLO, THI=THI, T=T, have_b1=have_b1)
    return in_maps, meta


def build_program(meta, ncores=8, act=GELU):
    NA, NW, NPAD = meta["NA"], meta["NW"], meta["NPAD"]
    TLO, THI, T = meta["TLO"], meta["THI"], meta["T"]
    have_b1 = meta["have_b1"]
    EPW = T * H

    nc = bacc.Bacc("TRN2", target_bir_lowering=False, debug=False,
                   num_devices=ncores)
    xT = nc.dram_tensor("xT", [H, NA], F16, kind="ExternalInput").ap()
    xfm = nc.dram_tensor("xfm", [H, NPAD], F32, kind="ExternalInput").ap()
    basd = nc.dram_tensor("basd", [NW * 128, EPW], F16,
                          kind="ExternalInput").ap()
    sd = nc.dram_tensor("sd", [NW * 128, EPW], F16,
                        kind="ExternalInput").ap()
    ilod = nc.dram_tensor("ilod", [NW * 128, TLO * 8], I16,
                          kind="ExternalInput").ap()
    ihid = (nc.dram_tensor("ihid", [NW * 128, THI * 8], I16,
                           kind="ExternalInput").ap() if THI else None)
    w1 = nc.dram_tensor("w1", [H, H], F16, kind="ExternalInput").ap()
    w2 = nc.dram_tensor("w2", [H, H], F16, kind="ExternalInput").ap()
    w3 = nc.dram_tensor("w3", [H, H], F16, kind="ExternalInput").ap()
    c2 = nc.dram_tensor("c2", [H, 1], F32, kind="ExternalInput").ap()
    c3 = nc.dram_tensor("c3", [H, 1], F32, kind="ExternalInput").ap()
    b1 = (nc.dram_tensor("b1", [1, H], F16, kind="ExternalInput").ap()
          if have_b1 else None)
    outd = nc.dram_tensor("out", [H, NPAD], F32, kind="ExternalOutput").ap()
    xp_h = nc.dram_tensor("xp", [NA, H], F16)
    xp_full = xp_h.ap()

    with tile.TileContext(nc) as tc:
        with (
            tc.tile_pool(name="const", bufs=1) as cpool,
            tc.tile_pool(name="xa", bufs=3) as xa,
            tc.tile_pool(name="xo", bufs=3) as xo,
            tc.tile_pool(name="pa", bufs=2, space="PSUM") as pa,
            tc.tile_pool(name="bas", bufs=3) as basp,
            tc.tile_pool(name="gat", bufs=3) as gatp,
            tc.tile_pool(name="st", bufs=3) as stp,
            tc.tile_pool(name="small", bufs=3) as smallp,
            tc.tile_pool(name="ffn", bufs=2) as ffnp,
            tc.tile_pool(name="pag", bufs=2, space="PSUM") as pag,
            tc.tile_pool(name="pffn", bufs=2, space="PSUM") as pffn,
        ):
            # constants
            w1t = cpool.tile([H, H], F16, tag="w1")
            nc.sync.dma_start(w1t[:], w1[:])
            w2t = cpool.tile([H, H], F16, tag="w2")
            nc.sync.dma_start(w2t[:], w2[:])
            w3t = cpool.tile([H, H], F16, tag="w3")
            nc.sync.dma_start(w3t[:], w3[:])
            c2t = cpool.tile([H, 1], F32, tag="c2")
            nc.sync.dma_start(c2t[:], c2[:])
            c3t = cpool.tile([H, 1], F32, tag="c3")
            nc.sync.dma_start(c3t[:], c3[:])
            if have_b1:
                b1t = cpool.tile([1, H], F16, tag="b1")
                nc.sync.dma_start(b1t[:], b1[:])
                onest = cpool.tile([1, H], F16, tag="ones")
                nc.gpsimd.memset(onest[:], 1.0)

            # ---- Phase A: xp = gelu(x @ W1 [+ b1]), node-major fp16 ----
            CH = 512
            for c0 in range(0, NA, CH):
                cols = min(CH, NA - c0)
                nb = cols // 128
                xt_t = xa.tile([H, CH], F16, tag="xa")
                nc.sync.dma_start(xt_t[:, :cols], xT[:, c0:c0 + cols])
                ps = pa.tile([128, CH], F32, tag="pa")
                for b in range(nb):
                    nc.tensor.matmul(
                        ps[:, b * 128:(b + 1) * 128],
                        xt_t[:, b * 128:(b + 1) * 128],
                        w1t[:],
                        start=True, stop=not have_b1)
                    if have_b1:
                        nc.tensor.matmul(
                            ps[:, b * 128:(b + 1) * 128],
                            onest[:1, :], b1t[:1, :],
                            start=False, stop=True)
                xo_t = xo.tile([128, CH], F16, tag="xo")
                nc.scalar.activation(xo_t[:, :cols], ps[:, :cols], act)
                dst_ap = bass.AP(xp_h, c0 * H,
                                 [[H, 128], [128 * H, nb], [1, H]])
                nc.sync.dma_start(dst_ap, xo_t[:, :cols])

            # ---- Phase B: gather, multiply, one-hot scatter, FFN ----
            for w in range(NW):
                r0 = w * 128
                bas_t = basp.tile([128, EPW], F16, tag="bas")
                nc.scalar.dma_start(bas_t[:], basd[r0:r0 + 128, :])
                il_t = smallp.tile([128, TLO * 8], I16, tag="il")
                nc.scalar.dma_start(il_t[:], ilod[r0:r0 + 128, :])
                if THI:
                    ih_t = smallp.tile([128, THI * 8], I16, tag="ih")
                    nc.scalar.dma_start(ih_t[:], ihid[r0:r0 + 128, :])
                s_t = stp.tile([128, EPW], F16, tag="s")
                nc.sync.dma_start(s_t[:], sd[r0:r0 + 128, :])

                g_t = gatp.tile([128, EPW], F16, tag="gat")
                g3 = g_t[:].rearrange("p (t f) -> p t f", f=H)
                nc.gpsimd.dma_gather(g3[:, 0:TLO, :], xp_full, il_t[:],
                                     TLO * 128, TLO * 128, H,
                                     single_packet=False)
                if THI:
                    nc.gpsimd.dma_gather(g3[:, TLO:T, :],
                                         xp_full[SPLIT:NA, :], ih_t[:],
                                         THI * 128, THI * 128, H,
                                         single_packet=False)
                msg_t = gatp.tile([128, EPW], F16, tag="msg")
                nc.vector.tensor_mul(msg_t[:], g_t[:], bas_t[:])

                ps_ag = pag.tile([128, 128], F32, tag="pag")
                for t in range(T):
                    nc.tensor.matmul(
                        ps_ag[:],
                        msg_t[:, t * 128:(t + 1) * 128],
                        s_t[:, t * 128:(t + 1) * 128],
                        start=(t == 0), stop=(t == T - 1))

                xf_t = smallp.tile([128, 128], F32, tag="xf")
                nc.scalar.dma_start(xf_t[:], xfm[:, r0:r0 + 128])
                x32_t = ffnp.tile([128, 128], F32, tag="x32")
                nc.vector.tensor_add(x32_t[:], ps_ag[:], xf_t[:])
                x16_t = ffnp.tile([128, 128], F16, tag="x16")
                nc.vector.tensor_copy(x16_t[:], x32_t[:])

                ps2 = pffn.tile([128, 128], F32, tag="pffn")
                nc.tensor.matmul(ps2[:], w2t[:], x16_t[:],
                                 start=True, stop=True)
                y1_t = ffnp.tile([128, 128], F16, tag="y1")
                nc.scalar.activation(y1_t[:], ps2[:], act, bias=c2t[:, 0:1])
                ps3 = pffn.tile([128, 128], F32, tag="pffn")
                nc.tensor.matmul(ps3[:], w3t[:], y1_t[:],
                                 start=True, stop=True)
                y2_t = ffnp.tile([128, 128], F32, tag="y2")
                nc.scalar.activation(y2_t[:], ps3[:], act, bias=c3t[:, 0:1])
                o_t = ffnp.tile([128, 128], F32, tag="o")
                nc.vector.tensor_add(o_t[:], y2_t[:], x32_t[:])
                nc.sync.dma_start(outd[:, r0:r0 + 128], o_t[:])

    nc.compile()
    return nc


def run_compiled(nc, in_maps, meta, ncores=8, **kw):
    res = run_bass_kernel_spmd(nc, in_maps, list(range(ncores)), **kw)
    N, NSH = meta["N"], meta["NSH"]
    out = np.empty((N, H), np.float32)
    for k in range(ncores):
        out[k * NSH:(k + 1) * NSH] = res.results[k]["out"][:, :NSH].T
    return out, res


def kernel(**inputs):
    inputs = {k: np.asarray(v) for k, v in inputs.items()}
    in_maps, meta = prep_inputs(**inputs)
    nc = build_program(meta)
    out, _ = run_compiled(nc, in_maps, meta)
    return out
